# revision 11
# baseline (speedup 1.0000x reference)
"""Trainium2 Bass kernel for nn_CapsuleLayer (capsule conv + 3-iter routing).

Reference computation (per batch image, C=128, H=W=32, K=3, pad=1):
  priors[h,w,t,nc] = sum_c x_pad[c, h+i, w+j] * W[t, c, nc] + b[t, nc]
    (t = i*3+j over the 9 kernel taps; nc = cap*16 + cch, 32 caps x 16 cch)
  o = mean_t priors
  3x: d2[t,cap] = sum_cch (o - p_t)^2 ; cw = rsqrt(d2 + 1e-4)
      cw = cw / sum_t cw ; o = sum_t cw_t * p_t
  out[nc, h, w] = o

Sharding: data-parallel over batch, 8 cores, one image each. Weight/bias
replicated. No collectives.

SBUF layouts per core:
  xcol   [128c, 9t, 1024pos]  im2col: per-tap shifted padded image
  W      [128c, 9t, 512nc]
  priors [128pos, 9t, 32cap, 16cch]  per 128-position chunk (8 chunks)
  o      [128pos, 32cap, 16cch]
Final: PE-transpose o into [nc, pos] blocks -> SBUF -> DRAM out.
"""

import numpy as np

C = 128
H = W = 32
B = 8
KK = 9  # 3x3 taps
NCAPS = 32
CCH = 16
NC = NCAPS * CCH  # 512
NIT = 3
NPOS = H * W  # 1024
CHUNK = 128  # positions per chunk (4 output rows)
NCHUNK = NPOS // CHUNK  # 8
PADW = 34  # padded width

_cache = {}


def _build(with_bias: bool):
    import concourse.bass as bass
    import concourse.tile as tile
    from concourse import bacc, mybir
    from concourse.masks import make_identity

    f32 = mybir.dt.float32
    X = mybir.AxisListType.X
    ADD = mybir.AluOpType.add

    nc = bacc.Bacc()
    x_d = nc.dram_tensor("x", [C, H, W], f32, kind="ExternalInput")
    w_d = nc.dram_tensor("w", [KK, C, NC], f32, kind="ExternalInput")
    b_d = nc.dram_tensor("b", [KK, NC], f32, kind="ExternalInput")
    out_d = nc.dram_tensor("out", [NC, NPOS], f32, kind="ExternalOutput")

    with tile.TileContext(nc) as tc:
        with (
            tc.tile_pool(name="singles", bufs=1) as singles,
            tc.tile_pool(name="priors", bufs=2) as priors_pool,
            tc.tile_pool(name="big", bufs=2) as big_pool,
            tc.tile_pool(name="o", bufs=2) as o_pool,
            tc.tile_pool(name="small", bufs=4) as small_pool,
            tc.tile_pool(name="pp", bufs=6, space="PSUM") as pp,
            tc.tile_pool(name="tpp", bufs=2, space="PSUM") as tpp,
        ):
            # ---- constants / inputs staged in SBUF ----
            xpad = singles.tile([C, PADW * PADW], f32)
            nc.gpsimd.memset(xpad, 0.0)
            xpad_v = xpad[:].rearrange("p (h w) -> p h w", h=PADW)
            nc.sync.dma_start(out=xpad_v[:, 1 : H + 1, 1 : W + 1], in_=x_d[:])

            # im2col: xcol[t][:, h*32+w] = xpad[:, h+i, w+j]
            xcol = []
            for t in range(KK):
                i, j = divmod(t, 3)
                xc = singles.tile([C, NPOS], f32, tag=f"xcol{t}")
                nc.sync.dma_start(
                    out=xc[:].rearrange("p (h w) -> p h w", h=H),
                    in_=xpad_v[:, i : i + H, j : j + W],
                )
                xcol.append(xc)

            wsb = singles.tile([C, KK, NC], f32)
            nc.sync.dma_start(out=wsb[:], in_=w_d[:].transpose([1, 0, 2]))

            ident = singles.tile([128, 128], f32)
            make_identity(nc, ident[:])

            eps = singles.tile([128, 1], f32)
            nc.gpsimd.memset(eps, 1e-4)

            if with_bias:
                bsb = singles.tile([1, KK, NC], f32)
                nc.sync.dma_start(out=bsb[:], in_=b_d[:].unsqueeze(0))
                ones = singles.tile([1, CHUNK], f32)
                nc.gpsimd.memset(ones, 1.0)

            for ch in range(NCHUNK):
                priors = priors_pool.tile([128, KK, NCAPS, CCH], f32)
                for t in range(KK):
                    ps = pp.tile([128, NC], f32)
                    lhsT = xcol[t][:, CHUNK * ch : CHUNK * (ch + 1)]
                    rhs = wsb[:, t, :]
                    if with_bias:
                        nc.tensor.matmul(ps[:], lhsT, rhs, start=True, stop=False)
                        nc.tensor.matmul(
                            ps[:], ones[:], bsb[:, t, :], start=False, stop=True
                        )
                    else:
                        nc.tensor.matmul(ps[:], lhsT, rhs, start=True, stop=True)
                    nc.scalar.copy(
                        out=priors[:, t],
                        in_=ps[:].rearrange("p (a b) -> p a b", a=NCAPS),
                    )

                # ---- routing ----
                o = o_pool.tile([128, NCAPS, CCH], f32)
                nc.vector.tensor_reduce(
                    out=o[:],
                    in_=priors[:].transpose([0, 2, 3, 1]),
                    axis=X,
                    op=ADD,
                )
                nc.scalar.mul(o[:], o[:], 1.0 / KK)

                d = big_pool.tile([128, KK, NCAPS, CCH], f32)
                for it in range(NIT):
                    ob = o[:].unsqueeze(1).broadcast_to((128, KK, NCAPS, CCH))
                    nc.vector.tensor_sub(d[:], priors[:], ob)
                    nc.vector.tensor_mul(d[:], d[:], d[:])
                    dist = small_pool.tile([128, KK, NCAPS], f32)
                    nc.vector.tensor_reduce(out=dist[:], in_=d[:], axis=X, op=ADD)
                    # cw = 1/sqrt(dist + eps)
                    nc.scalar.activation(
                        out=dist[:],
                        in_=dist[:],
                        func=mybir.ActivationFunctionType.Sqrt,
                        bias=eps[:],
                    )
                    cw = small_pool.tile([128, KK, NCAPS], f32)
                    nc.vector.reciprocal(cw[:], dist[:])
                    cwsum = small_pool.tile([128, NCAPS], f32)
                    nc.vector.tensor_reduce(
                        out=cwsum[:],
                        in_=cw[:].transpose([0, 2, 1]),
                        axis=X,
                        op=ADD,
                    )
                    rs = small_pool.tile([128, NCAPS], f32)
                    nc.vector.reciprocal(rs[:], cwsum[:])
                    nc.vector.tensor_mul(
                        cw[:], cw[:], rs[:].unsqueeze(1).broadcast_to((128, KK, NCAPS))
                    )
                    # weighted sum of priors
                    cwb = cw[:].unsqueeze(3).broadcast_to((128, KK, NCAPS, CCH))
                    nc.vector.tensor_mul(d[:], priors[:], cwb)
                    o = o_pool.tile([128, NCAPS, CCH], f32)
                    nc.vector.tensor_reduce(
                        out=o[:],
                        in_=d[:].transpose([0, 2, 3, 1]),
                        axis=X,
                        op=ADD,
                    )

                # ---- transpose [pos, (cap,cch)] -> [nc, pos] and store ----
                o_flat = o[:].rearrange("p a b -> p (a b)")
                for blk in range(4):
                    tp = tpp.tile([128, 128], f32)
                    nc.tensor.transpose(
                        tp[:], o_flat[:, 128 * blk : 128 * (blk + 1)], ident[:]
                    )
                    ot = small_pool.tile([128, 128], f32, tag="ostage")
                    nc.scalar.copy(out=ot[:], in_=tp[:])
                    nc.sync.dma_start(
                        out=out_d[
                            128 * blk : 128 * (blk + 1), 128 * ch : 128 * (ch + 1)
                        ],
                        in_=ot[:],
                    )
    nc.compile()
    return nc


def _get_nc(with_bias: bool):
    key = ("nc", with_bias)
    if key not in _cache:
        _cache[key] = _build(with_bias)
    return _cache[key]


def kernel(input, weight, bias, _trace=False):
    from concourse.bass_utils import run_bass_kernel_spmd

    input = np.ascontiguousarray(np.asarray(input, dtype=np.float32))
    w = np.ascontiguousarray(
        np.asarray(weight, dtype=np.float32).reshape(KK, C, NC)
    )
    b = np.ascontiguousarray(np.asarray(bias, dtype=np.float32).reshape(KK, NC))
    with_bias = bool(np.any(b))

    nc = _get_nc(with_bias)
    in_maps = [
        {"x": np.ascontiguousarray(input[i]), "w": w, "b": b} for i in range(B)
    ]
    res = run_bass_kernel_spmd(
        nc, in_maps, core_ids=list(range(B)), trace=_trace
    )
    _cache["last_result"] = res
    out = np.stack(
        [r["out"].reshape(NC, H, W) for r in res.results], axis=0
    )
    return out


# revision 15
# speedup vs baseline: 1.8869x; 1.8869x over previous
"""Trainium2 Bass kernel for nn_CapsuleLayer (capsule conv + 3-iter routing).

Reference (per batch image, C=128, H=W=32, K=3, pad=1):
  priors[h,w,t,nc] = sum_c x_pad[c,h+i,w+j] * W[t,c,nc] + b[t,nc]
  o = mean_t priors
  3x: d2 = sum_cch (o - p_t)^2 ; cw = rsqrt(d2 + 1e-4)
      cw = cw / sum_t cw ; o = sum_t cw_t p_t
  out[nc,h,w] = o

Sharding: data-parallel over batch; 8 cores, one image each; weight/bias
replicated; no collectives.

Implementation notes (v2):
- bf16 everywhere on-chip (fp32 PSUM accumulation in matmuls); rel err vs
  fp32 reference ~5e-3, within the 2e-2 gate.
- priors layout [128pos, tap9, cch16, cap32] so every big DVE op runs in
  2x_1P mode (innermost step-1 cap runs, broadcasts on outer/mid dims).
- ||o - p||^2 = ||p||^2 - <2o, p> + ||o||^2: the only per-iteration
  full-size passes are the product o2*p, its cch-reduction, the weighted
  product p*alpha and its tap-reduction. Reductions are halving adds
  (2x mode) instead of 1x tensor_reduce.
- mean_t priors comes from 9 extra accumulating matmuls on the idle PE.
- rsqrt via exp(-0.5*ln(x)) on the Scalar engine (sanctioned rsqrt path
  is DVE-reciprocal-heavy; cw is scale-invariant after normalization).
- ||o||^2 = sum_t alpha_t <o, p_t> reuses the s-values (no extra pass).
"""

import numpy as np

C = 128
H = W = 32
B = 8
KK = 9
NCAPS = 32
CCH = 16
NC = NCAPS * CCH  # 512
NIT = 3
NPOS = H * W
CHUNK = 128
NCHUNK = NPOS // CHUNK
PADW = 34

_cache = {}


def _build(with_bias: bool):
    import concourse.bass as bass
    import concourse.tile as tile
    from concourse import bacc, mybir
    from concourse.masks import make_identity

    f32 = mybir.dt.float32
    bf16 = mybir.dt.bfloat16
    X = mybir.AxisListType.X
    ADD = mybir.AluOpType.add
    AF = mybir.ActivationFunctionType

    nc = bacc.Bacc()
    x_d = nc.dram_tensor("x", [C, H, W], f32, kind="ExternalInput")
    w_d = nc.dram_tensor("w", [KK, C, NC], f32, kind="ExternalInput")
    b_d = nc.dram_tensor("b", [KK, NC], f32, kind="ExternalInput")
    out_d = nc.dram_tensor("out", [NC, NPOS], f32, kind="ExternalOutput")

    with tile.TileContext(nc) as tc:
        with (
            tc.tile_pool(name="singles", bufs=1) as singles,
            tc.tile_pool(name="priors", bufs=2) as priors_pool,
            tc.tile_pool(name="big", bufs=2) as big_pool,
            tc.tile_pool(name="half", bufs=2) as half_pool,
            tc.tile_pool(name="o", bufs=2) as o_pool,
            tc.tile_pool(name="small", bufs=4) as small_pool,
            tc.tile_pool(name="pp", bufs=4, space="PSUM") as pp,
            tc.tile_pool(name="mp", bufs=2, space="PSUM") as mp,
            tc.tile_pool(name="tpp", bufs=2, space="PSUM") as tpp,
        ):
            # ---- stage inputs: pad, cast to bf16, im2col, permute W ----
            xpadf = singles.tile([C, PADW * PADW], f32)
            nc.gpsimd.memset(xpadf, 0.0)
            xpadf_v = xpadf[:].rearrange("p (h w) -> p h w", h=PADW)
            nc.sync.dma_start(out=xpadf_v[:, 1 : H + 1, 1 : W + 1], in_=x_d[:])
            xpad = singles.tile([C, PADW * PADW], bf16)
            nc.scalar.copy(out=xpad[:], in_=xpadf[:])
            xpad_v = xpad[:].rearrange("p (h w) -> p h w", h=PADW)

            xcol = []
            for t in range(KK):
                i, j = divmod(t, 3)
                xc = singles.tile([C, NPOS], bf16, tag=f"xcol{t}")
                nc.sync.dma_start(
                    out=xc[:].rearrange("p (h w) -> p h w", h=H),
                    in_=xpad_v[:, i : i + H, j : j + W],
                )
                xcol.append(xc)

            wraw = singles.tile([C, KK, NC], f32)
            nc.sync.dma_start(out=wraw[:], in_=w_d[:].transpose([1, 0, 2]))
            # wsb[c, t, cch, cap] (bf16) <- wraw[c, t, cap*16+cch]
            wsb = singles.tile([C, KK, CCH, NCAPS], bf16)
            nc.scalar.copy(
                out=wsb[:],
                in_=wraw[:].rearrange("p t (cap cch) -> p t cch cap", cch=CCH),
            )

            ident = singles.tile([128, 128], f32)
            make_identity(nc, ident[:])

            if with_bias:
                braw = singles.tile([1, KK, NC], f32)
                nc.sync.dma_start(out=braw[:], in_=b_d[:].unsqueeze(0))
                bsb = singles.tile([1, KK, CCH, NCAPS], bf16)
                nc.scalar.copy(
                    out=bsb[:],
                    in_=braw[:].rearrange("p t (cap cch) -> p t cch cap", cch=CCH),
                )
                ones = singles.tile([1, CHUNK], bf16)
                nc.gpsimd.memset(ones, 1.0)

            for ch in range(NCHUNK):
                # ---- priors + mean via PE ----
                priors = priors_pool.tile([128, KK, CCH, NCAPS], bf16)
                om = mp.tile([128, NC], f32)  # sum_t priors (fp32 psum)
                for t in range(KK):
                    ps = pp.tile([128, NC], f32)
                    lhsT = xcol[t][:, CHUNK * ch : CHUNK * (ch + 1)]
                    rhs = wsb[:, t].rearrange("p a b -> p (a b)")
                    if with_bias:
                        nc.tensor.matmul(ps[:], lhsT, rhs, start=True, stop=False)
                        brhs = bsb[:, t].rearrange("p a b -> p (a b)")
                        nc.tensor.matmul(ps[:], ones[:], brhs, start=False, stop=True)
                    else:
                        nc.tensor.matmul(ps[:], lhsT, rhs, start=True, stop=True)
                    nc.tensor.matmul(
                        om[:], lhsT, rhs, start=(t == 0), stop=(t == KK - 1)
                    )
                    if with_bias:
                        # mean accumulates bias once per tap as well
                        nc.tensor.matmul(
                            om[:], ones[:], brhs, start=False, stop=False,
                            skip_group_check=True,
                        )
                    nc.scalar.copy(
                        out=priors[:, t],
                        in_=ps[:].rearrange("p (a b) -> p a b", a=CCH),
                    )

                # o2 = 2*mean = (2/9) * sum_t priors   [128, (cch,cap)] bf16
                o2 = o_pool.tile([128, NC], bf16)
                nc.scalar.activation(
                    out=o2[:], in_=om[:], func=AF.Copy, scale=2.0 / KK
                )

                # ---- n[t,cap] = sum_cch p^2 (one-time per chunk) ----
                tprod = big_pool.tile([128, KK, CCH, NCAPS], bf16)
                nc.scalar.activation(out=tprod[:], in_=priors[:], func=AF.Square)
                h1 = half_pool.tile([128, KK, 8, NCAPS], bf16)
                nc.vector.tensor_add(
                    h1[:], tprod[:, :, 0:8, :], tprod[:, :, 8:16, :]
                )
                nc.vector.tensor_add(h1[:, :, 0:4], h1[:, :, 0:4], h1[:, :, 4:8])
                nc.vector.tensor_add(h1[:, :, 0:2], h1[:, :, 0:2], h1[:, :, 2:4])
                ntile = small_pool.tile([128, KK, NCAPS], bf16, tag="n")
                nc.vector.tensor_add(ntile[:], h1[:, :, 0], h1[:, :, 1])

                alpha = None  # bf16 [128, KK, NCAPS]; None => uniform 1/9
                for it in range(NIT):
                    last = it == NIT - 1
                    # s = <o2, p_t> per (tap, cap): product + cch halving
                    tprod = big_pool.tile([128, KK, CCH, NCAPS], bf16)
                    ob = (
                        o2[:]
                        .rearrange("p (a b) -> p a b", a=CCH)
                        .unsqueeze(1)
                        .broadcast_to((128, KK, CCH, NCAPS))
                    )
                    nc.vector.tensor_mul(tprod[:], priors[:], ob)
                    h1 = half_pool.tile([128, KK, 8, NCAPS], bf16)
                    nc.vector.tensor_add(
                        h1[:], tprod[:, :, 0:8, :], tprod[:, :, 8:16, :]
                    )
                    nc.vector.tensor_add(
                        h1[:, :, 0:4], h1[:, :, 0:4], h1[:, :, 4:8]
                    )
                    nc.vector.tensor_add(
                        h1[:, :, 0:2], h1[:, :, 0:2], h1[:, :, 2:4]
                    )
                    s = small_pool.tile([128, KK, NCAPS], bf16, tag="s")
                    nc.vector.tensor_add(s[:], h1[:, :, 0], h1[:, :, 1])

                    # e2 = sum_t alpha_t * s_t ; e = ||o||^2 (+eps folded)
                    e2 = small_pool.tile([128, NCAPS], f32, tag="e2")
                    if alpha is None:
                        nc.vector.tensor_reduce(
                            out=e2[:], in_=s[:].transpose([0, 2, 1]), axis=X, op=ADD
                        )
                        # e' = e2/(2*9) + eps
                        nc.vector.tensor_scalar(
                            e2[:], e2[:], 1.0 / (2 * KK), 1e-4,
                            op0=mybir.AluOpType.mult, op1=ADD,
                        )
                    else:
                        tm = small_pool.tile([128, KK, NCAPS], f32, tag="tm")
                        nc.vector.tensor_mul(tm[:], alpha[:], s[:])
                        nc.vector.tensor_reduce(
                            out=e2[:], in_=tm[:].transpose([0, 2, 1]), axis=X, op=ADD
                        )
                        # o2 = sum alpha2*p with alpha2 = 2*alpha_norm
                        # => e = ||o||^2 = (1/4) sum alpha2 <o2, p> = e2/4
                        nc.vector.tensor_scalar(
                            e2[:], e2[:], 0.25, 1e-4,
                            op0=mybir.AluOpType.mult, op1=ADD,
                        )

                    # dist = (n - s) + e'  (fp32)
                    dist = small_pool.tile([128, KK, NCAPS], f32, tag="dist")
                    nc.vector.tensor_sub(dist[:], ntile[:], s[:])
                    nc.vector.tensor_add(
                        dist[:],
                        dist[:],
                        e2[:].unsqueeze(1).broadcast_to((128, KK, NCAPS)),
                    )
                    # cwu = dist^-0.5 = exp(-0.5*ln(dist)) on ACT
                    nc.scalar.activation(out=dist[:], in_=dist[:], func=AF.Ln)
                    cwu = small_pool.tile([128, KK, NCAPS], bf16, tag="cwu")
                    nc.scalar.activation(
                        out=cwu[:], in_=dist[:], func=AF.Exp, scale=-0.5
                    )
                    # alpha = cwu / sum_t cwu  (doubled except last iter)
                    cwsum = small_pool.tile([128, NCAPS], f32, tag="cwsum")
                    nc.vector.tensor_reduce(
                        out=cwsum[:], in_=cwu[:].transpose([0, 2, 1]), axis=X, op=ADD
                    )
                    rs = small_pool.tile([128, NCAPS], f32, tag="rs")
                    nc.vector.reciprocal(rs[:], cwsum[:])
                    if not last:
                        nc.vector.tensor_scalar_mul(rs[:], rs[:], 2.0)
                    alpha = small_pool.tile([128, KK, NCAPS], bf16, tag="alpha")
                    nc.vector.tensor_mul(
                        alpha[:],
                        cwu[:],
                        rs[:].unsqueeze(1).broadcast_to((128, KK, NCAPS)),
                    )

                    # o' = sum_t alpha_t p_t : product + tap halving
                    wprod = big_pool.tile([128, KK, CCH, NCAPS], bf16, tag="wp")
                    ab = alpha[:].unsqueeze(2).broadcast_to((128, KK, CCH, NCAPS))
                    nc.vector.tensor_mul(wprod[:], priors[:], ab)
                    wp = wprod[:].rearrange("p t a b -> p t (a b)")
                    wh = half_pool.tile([128, 4, NC], bf16, tag="wh")
                    nc.vector.tensor_add(wh[:], wp[:, 0:4], wp[:, 4:8])
                    nc.vector.tensor_add(wh[:, 0:2], wh[:, 0:2], wh[:, 2:4])
                    nc.vector.tensor_add(wh[:, 0], wh[:, 0], wh[:, 1])
                    if not last:
                        o2 = o_pool.tile([128, NC], bf16)
                        nc.vector.tensor_add(o2[:], wh[:, 0], wp[:, 8])
                    else:
                        # write final o in natural (cap, cch) order, fp32
                        onat = o_pool.tile([128, NC], f32, tag="onat")
                        nc.vector.tensor_add(
                            onat[:].rearrange("p (cap cch) -> p cch cap", cch=CCH),
                            wh[:, 0].rearrange("p (cch cap) -> p cch cap", cch=CCH),
                            wp[:, 8].rearrange("p (cch cap) -> p cch cap", cch=CCH),
                        )

                # ---- transpose to [nc, pos] and store ----
                for blk in range(4):
                    tp = tpp.tile([128, 128], f32)
                    nc.tensor.transpose(
                        tp[:], onat[:, 128 * blk : 128 * (blk + 1)], ident[:]
                    )
                    ot = small_pool.tile([128, 128], f32, tag="ostage")
                    nc.scalar.copy(out=ot[:], in_=tp[:])
                    nc.sync.dma_start(
                        out=out_d[
                            128 * blk : 128 * (blk + 1), 128 * ch : 128 * (ch + 1)
                        ],
                        in_=ot[:],
                    )
    nc.compile()
    return nc


def _get_nc(with_bias: bool):
    key = ("nc", with_bias)
    if key not in _cache:
        _cache[key] = _build(with_bias)
    return _cache[key]


def kernel(input, weight, bias, _trace=False):
    from concourse.bass_utils import run_bass_kernel_spmd

    input = np.ascontiguousarray(np.asarray(input, dtype=np.float32))
    w = np.ascontiguousarray(
        np.asarray(weight, dtype=np.float32).reshape(KK, C, NC)
    )
    b = np.ascontiguousarray(np.asarray(bias, dtype=np.float32).reshape(KK, NC))
    with_bias = bool(np.any(b))

    nc = _get_nc(with_bias)
    in_maps = [
        {"x": np.ascontiguousarray(input[i]), "w": w, "b": b} for i in range(B)
    ]
    res = run_bass_kernel_spmd(
        nc, in_maps, core_ids=list(range(B)), trace=_trace
    )
    _cache["last_result"] = res
    out = np.stack(
        [r["out"].reshape(NC, H, W) for r in res.results], axis=0
    )
    return out
